# revision 9
# baseline (speedup 1.0000x reference)
"""Trainium2 Bass kernel for the controlled-U (CU) gate application.

Math: the reference builds U = P0 (x) I (x) ... + P1 (x) Mexp (x) I ...
with dim=2, wires=12, index=(0,1), control_state=(1,). This factors as

    U = diag(I_2048, Mexp (x) I_1024)        (4096 x 4096)

so U @ x is:
    out[0:2048]     = x[0:2048]                        (identity)
    out[2048:3072]  = c00 * x[2048:3072] + c01 * x[3072:4096]
    out[3072:4096]  = c10 * x[2048:3072] + c11 * x[3072:4096]

with [[c00, c01], [c10, c11]] = Mexp = expm(M - M^H), a 2x2 unitary
computed exactly on host (eigendecomposition of the 2x2 Hermitian
generator).

The identity block is pure data movement with zero arithmetic, so it is
handled in the host-side gather: the top 2048 output rows are assembled
directly from the input array while interleaving to complex64 (the host
touches every element there anyway).  The device computes only the
non-trivial part -- the 2x2 complex mix over the bottom 2048 rows --
sharded row-wise across the 8 cores (128 pair-rows each).

Device-side formulation: each core packs its slice as four 128-partition
tiles where partitions carry 32-row groups of (x1_re, x1_im, x2_re,
x2_im) for paired rows (x1 from the c00/c01 block, x2 from the c10/c11
block).  The whole complex 2x2 mix is then ONE bf16 matmul per
512-column PSUM bank with a single constant stationary

    V = kron(Q, I_32),   Q[a, b] = coefficient of input group a
                                   in output group b

(out = V^T @ x contracts the partition dim, mixing re/im and the two
row blocks in one pass).  8 matmuls + 1 stationary load total; PSUM
banks are evacuated to bf16 SBUF split between the ACT and DVE engines,
and the 4 tile stores alternate the two HWDGE rings so loads and stores
overlap.  Per-core HBM traffic is ~2.06 MiB (vs 4.2 MiB when the
identity block rode the device), and the bf16 pipeline measures
~4e-3 rel err against the 2e-2 gate.
"""

import ml_dtypes
import numpy as np

import bass_rust
import concourse.bacc as bacc
import concourse.mybir as mybir
from concourse.tile import TileContext
from concourse.bass_utils import run_bass_kernel_spmd

# Problem geometry (hardcoded per the task contract).
D = 4096           # state dimension 2**12
B = 1024           # batch
NCORES = 8
P = 128            # SBUF partitions
PROWS = D // 4 // NCORES   # 128 bottom pair rows per core
NT = 4             # tiles per core: 4 x [128, 1024]
CH = 512           # PSUM bank = 512 f32 columns
G = 32             # rows per partition group (4 groups of 32 = 128)
F32 = mybir.dt.float32
BF16 = mybir.dt.bfloat16
NPBF = ml_dtypes.bfloat16

N_WARM = 16        # dummy matmuls to unthrottle the PE clock gate
PREP_LAST_TILE = True


def _build_nc() -> bacc.Bacc:
    """Build the per-core Bass/Tile program (identical on all 8 cores)."""
    nc = bacc.Bacc("TRN2", enable_partition_id=False)

    v_in = nc.dram_tensor("V", [P, P], BF16, kind="ExternalInput")
    x_in = nc.dram_tensor("X", [P, NT * B], BF16, kind="ExternalInput")
    y_out = nc.dram_tensor("Y", [P, NT * B], BF16, kind="ExternalOutput")

    with TileContext(nc) as tc:
        with (
            tc.tile_pool(name="const", bufs=1) as const_pool,
            tc.tile_pool(name="io", bufs=1) as io_pool,
            tc.tile_pool(name="scr", bufs=1) as scr_pool,
            tc.tile_pool(name="psum", bufs=1, space="PSUM") as psum_pool,
        ):
            v_sb = const_pool.tile([P, P], BF16, tag="v")
            x_sbs = [io_pool.tile([P, B], BF16, tag=f"x{t}",
                                  name=f"x{t}_sb")
                     for t in range(NT)]
            y_sb = io_pool.tile([P, NT * B], BF16, tag="y")

            # Loads, split across the two HWDGE rings.  DMA transfers
            # effectively serialize through the shared DMA-engine pool at
            # ~360 GB/s, so what matters is descriptor-ready order: ring
            # SP carries tiles 0/1, ring ACT carries V (tiny) then tiles
            # 2/3 -- pool arrival order == consumption order while both
            # rings' issue costs overlap.
            nc.sync.dma_start(x_sbs[0][:], x_in[:, 0 * B : 1 * B])
            nc.scalar.dma_start(v_sb[:], v_in[:])
            nc.sync.dma_start(x_sbs[1][:], x_in[:, 1 * B : 2 * B])
            nc.scalar.dma_start(x_sbs[2][:], x_in[:, 2 * B : 3 * B])
            nc.scalar.dma_start(x_sbs[3][:], x_in[:, 3 * B : 4 * B])

            # PE warmup: the HAM clock gate runs the PE at 1.2 GHz until
            # it has seen ~4 us of sustained activity; dummy matmuls on a
            # memset tile bridge the load-latency window (first tile is
            # consumable ~4 us into the body) so payload matmuls run
            # closer to 2.4 GHz.  They write a payload PSUM bank --
            # harmless, the payload matmul resets it with start=True.
            dummy = scr_pool.tile([P, 2 * P], BF16, tag="dummy")
            nc.gpsimd.memset(dummy[:], 0.0)
            ctx0 = None
            if PREP_LAST_TILE:
                ctx0 = scr_pool.tile([P, 1], mybir.dt.int32, tag="ctx0")
                nc.gpsimd.memset(ctx0[:], 0)
                kv_sem = nc.alloc_semaphore("kvwb3")

            pts = [psum_pool.tile([P, CH], F32, tag=f"ps{k}", name=f"ps{k}")
                   for k in range(2 * NT)]
            for _ in range(N_WARM):
                nc.tensor.matmul(pts[0][:, 0 : 2 * P], dummy[:, 0:P],
                                 dummy[:], start=True, stop=True)

            # Payload: per tile, one matmul per 512-col half (its own
            # PSUM bank, single start/stop); ACT evacuates even banks,
            # DVE odd banks (bf16 cast).  Stores are SWDGE prepared
            # descriptors (one queue per tile): descriptors are generated
            # early on the idle gpsimd queue while loads stream, and each
            # trigger fires as soon as its tile's two evacs land -- the
            # ~1.9 us HWDGE issue latency never appears on the tail.
            # Preps sit AFTER their producer evacs in program order so
            # Tile's deferred-RAW machinery binds the evacs to the
            # trigger (validated in CoreSim; reversed order races).
            for t in range(NT):
                for h in range(2):
                    k = 2 * t + h
                    cs = slice(t * B + h * CH, t * B + (h + 1) * CH)
                    nc.tensor.matmul(pts[k][:], v_sb[:],
                                     x_sbs[t][:, h * CH : (h + 1) * CH],
                                     start=True, stop=True)
                    last_prep = PREP_LAST_TILE and t == NT - 1
                    if h == 0:
                        nc.scalar.copy(y_sb[:, cs], pts[k][:])
                        if not last_prep:
                            nc.sync.dma_start(y_out[:, cs], y_sb[:, cs])
                    else:
                        nc.vector.tensor_copy(y_sb[:, cs], pts[k][:])
                        if not last_prep:
                            nc.scalar.dma_start(y_out[:, cs], y_sb[:, cs])
                ts = slice(t * B, (t + 1) * B)
                if PREP_LAST_TILE and t == NT - 1:
                    # The final tile's store is a prepared SWDGE descriptor:
                    # desc-gen runs early on the idle gpsimd queue, and the
                    # trigger fires the moment both evacs land -- the ~1.9us
                    # HWDGE issue latency never lands on the critical tail.
                    in4 = y_sb[:, ts].rearrange("p (a b n) -> p a b n",
                                                a=1, b=1)
                    out4 = y_out[:, ts].unsqueeze(0).unsqueeze(2).copy()
                    out4.ap = bass_rust.VecI64Pair(
                        [(1, 1), (NT * B, P), (NT * B, 1), (1, B)])
                    nc.gpsimd.kv_writeback(out4, in4, ctx0[:],
                                           prepare_only=True, sem=kv_sem)
                    nc.gpsimd.trigger_dma(count=None)

    nc.finalize()
    return nc


_NC_CACHE = None


def _get_nc() -> bacc.Bacc:
    global _NC_CACHE
    if _NC_CACHE is None:
        _NC_CACHE = _build_nc()
    return _NC_CACHE


def _coef_values(M_re: np.ndarray, M_im: np.ndarray):
    """Host-side 2x2 expm of the anti-Hermitian generator -> V stationary.

    Returns (V, None): V is the [128, 128] bf16 kron(Q, I_32) stationary
    (second slot kept for interface compat with older harnesses).
    """
    M = M_re.astype(np.float64) + 1j * M_im.astype(np.float64)
    A = M - M.conj().T          # anti-Hermitian
    H = -1j * A                 # Hermitian
    w, Vec = np.linalg.eigh(H)
    Mexp = Vec @ np.diag(np.exp(1j * w)) @ Vec.conj().T   # expm(A), exact
    cr, ci = Mexp.real, Mexp.imag
    # out = V^T @ x with partition groups (x1re, x1im, x2re, x2im) and
    # output groups (o1re, o1im, o2re, o2im): V[p, i] = Q[p//32, i//32].
    Q = np.array([
        [cr[0, 0],  ci[0, 0],  cr[1, 0],  ci[1, 0]],
        [-ci[0, 0], cr[0, 0], -ci[1, 0],  cr[1, 0]],
        [cr[0, 1],  ci[0, 1],  cr[1, 1],  ci[1, 1]],
        [-ci[0, 1], cr[0, 1], -ci[1, 1],  cr[1, 1]],
    ], dtype=np.float32)
    V = np.kron(Q, np.eye(G, dtype=np.float32)).astype(NPBF)
    return V, None


def _in_map(x_re, x_im, V, cvec, d: int) -> dict:
    """Per-core input dict: pack the core's 128 pair-rows as 4 tiles of
    [128, 1024] with partition groups (x1re, x1im, x2re, x2im) x 32."""
    b1 = D // 2 + d * PROWS
    b2 = 3 * D // 4 + d * PROWS

    def grp(a, b0):
        return np.asarray(a[b0 : b0 + PROWS], dtype=NPBF).reshape(NT, G, B)

    # [NT, 4*G, B] -> [4*G, NT, B] -> [128, 4096]
    X = np.concatenate(
        [grp(x_re, b1), grp(x_im, b1), grp(x_re, b2), grp(x_im, b2)],
        axis=1,
    ).transpose(1, 0, 2).reshape(P, NT * B)
    return {"V": V, "X": np.ascontiguousarray(X)}


def kernel(M_re, M_im, x_re, x_im) -> np.ndarray:
    M_re = np.asarray(M_re, dtype=np.float32)
    M_im = np.asarray(M_im, dtype=np.float32)
    x_re = np.ascontiguousarray(x_re, dtype=np.float32)
    x_im = np.ascontiguousarray(x_im, dtype=np.float32)

    V, _ = _coef_values(M_re, M_im)
    in_maps = [_in_map(x_re, x_im, V, None, d) for d in range(NCORES)]

    nc = _get_nc()
    res = run_bass_kernel_spmd(nc, in_maps, core_ids=list(range(NCORES)))

    full = np.empty((D, B), dtype=np.complex64)
    # Identity block: assembled straight from the input during the gather.
    full.real[: D // 2] = x_re[: D // 2]
    full.imag[: D // 2] = x_im[: D // 2]
    for d, r in enumerate(res.results):
        b1 = D // 2 + d * PROWS
        b2 = 3 * D // 4 + d * PROWS
        Y = np.asarray(r["Y"]).reshape(P, NT, B).transpose(1, 0, 2)
        full.real[b1 : b1 + PROWS] = Y[:, 0 * G : 1 * G].reshape(PROWS, B)
        full.imag[b1 : b1 + PROWS] = Y[:, 1 * G : 2 * G].reshape(PROWS, B)
        full.real[b2 : b2 + PROWS] = Y[:, 2 * G : 3 * G].reshape(PROWS, B)
        full.imag[b2 : b2 + PROWS] = Y[:, 3 * G : 4 * G].reshape(PROWS, B)
    return full


# revision 10
# speedup vs baseline: 1.3642x; 1.3642x over previous
"""Trainium2 Bass kernel for the controlled-U (CU) gate application.

Math: the reference builds U = P0 (x) I (x) ... + P1 (x) Mexp (x) I ...
with dim=2, wires=12, index=(0,1), control_state=(1,). This factors as

    U = diag(I_2048, Mexp (x) I_1024)        (4096 x 4096)

so U @ x is:
    out[0:2048]     = x[0:2048]                        (identity)
    out[2048:3072]  = c00 * x[2048:3072] + c01 * x[3072:4096]
    out[3072:4096]  = c10 * x[2048:3072] + c11 * x[3072:4096]

with [[c00, c01], [c10, c11]] = Mexp = expm(M - M^H), a 2x2 unitary
computed exactly on host (eigendecomposition of the 2x2 Hermitian
generator).

The identity block is pure data movement with zero arithmetic, so it is
handled in the host-side gather: the top 2048 output rows are assembled
directly from the input array while interleaving to complex64 (the host
touches every element there anyway).  The device computes only the
non-trivial part -- the 2x2 complex mix over the bottom 2048 rows --
sharded row-wise across the 8 cores (128 pair-rows each).

Device-side formulation: each core packs its slice as four 128-partition
tiles where partitions carry 32-row groups of (x1_re, x1_im, x2_re,
x2_im) for paired rows (x1 from the c00/c01 block, x2 from the c10/c11
block).  The whole complex 2x2 mix is then ONE bf16 matmul per
512-column PSUM bank with a single constant stationary

    V = kron(Q, I_32),   Q[a, b] = coefficient of input group a
                                   in output group b

(out = V^T @ x contracts the partition dim, mixing re/im and the two
row blocks in one pass).  8 matmuls + 1 stationary load total; PSUM
banks are evacuated to bf16 SBUF split between the ACT and DVE engines,
and the 8 per-bank stores alternate the two HWDGE rings so loads and
stores overlap.  Per-core HBM traffic is ~2.06 MiB (vs 4.2 MiB when the
identity block rode the device), and the bf16 pipeline measures
~4e-3 rel err against the 2e-2 gate.
"""

import ml_dtypes
import numpy as np

import concourse.bacc as bacc
import concourse.mybir as mybir
from concourse.tile import TileContext
from concourse.bass_utils import run_bass_kernel_spmd

# Problem geometry (hardcoded per the task contract).
D = 4096           # state dimension 2**12
B = 1024           # batch
NCORES = 8
P = 128            # SBUF partitions
PROWS = D // 4 // NCORES   # 128 bottom pair rows per core
NT = 4             # tiles per core: 4 x [128, 1024]
CH = 512           # PSUM bank = 512 f32 columns
G = 32             # rows per partition group (4 groups of 32 = 128)
F32 = mybir.dt.float32
BF16 = mybir.dt.bfloat16
NPBF = ml_dtypes.bfloat16

N_WARM = 16        # dummy matmuls to unthrottle the PE clock gate


def _build_nc() -> bacc.Bacc:
    """Build the per-core Bass/Tile program (identical on all 8 cores)."""
    nc = bacc.Bacc("TRN2", enable_partition_id=False)

    v_in = nc.dram_tensor("V", [P, P], BF16, kind="ExternalInput")
    x_in = nc.dram_tensor("X", [P, NT * B], BF16, kind="ExternalInput")
    y_out = nc.dram_tensor("Y", [P, NT * B], BF16, kind="ExternalOutput")

    with TileContext(nc) as tc:
        with (
            tc.tile_pool(name="const", bufs=1) as const_pool,
            tc.tile_pool(name="io", bufs=1) as io_pool,
            tc.tile_pool(name="scr", bufs=1) as scr_pool,
            tc.tile_pool(name="psum", bufs=1, space="PSUM") as psum_pool,
        ):
            v_sb = const_pool.tile([P, P], BF16, tag="v")
            x_sbs = [io_pool.tile([P, B], BF16, tag=f"x{t}",
                                  name=f"x{t}_sb")
                     for t in range(NT)]
            y_sb = io_pool.tile([P, NT * B], BF16, tag="y")

            # Loads, split across the two HWDGE rings.  DMA transfers
            # effectively serialize through the shared DMA-engine pool at
            # ~360 GB/s, so what matters is descriptor-ready order: ring
            # SP carries tiles 0/1, ring ACT carries V (tiny) then tiles
            # 2/3 -- pool arrival order == consumption order while both
            # rings' issue costs overlap.
            nc.sync.dma_start(x_sbs[0][:], x_in[:, 0 * B : 1 * B])
            nc.scalar.dma_start(v_sb[:], v_in[:])
            nc.sync.dma_start(x_sbs[1][:], x_in[:, 1 * B : 2 * B])
            nc.scalar.dma_start(x_sbs[2][:], x_in[:, 2 * B : 3 * B])
            nc.scalar.dma_start(x_sbs[3][:], x_in[:, 3 * B : 4 * B])

            # PE warmup: the HAM clock gate runs the PE at 1.2 GHz until
            # it has seen ~4 us of sustained activity; dummy matmuls on a
            # memset tile bridge the load-latency window (first tile is
            # consumable ~4 us into the body) so payload matmuls run
            # closer to 2.4 GHz.  They write a payload PSUM bank --
            # harmless, the payload matmul resets it with start=True.
            dummy = scr_pool.tile([P, 2 * P], BF16, tag="dummy")
            nc.gpsimd.memset(dummy[:], 0.0)

            pts = [psum_pool.tile([P, CH], F32, tag=f"ps{k}", name=f"ps{k}")
                   for k in range(2 * NT)]
            for _ in range(N_WARM):
                nc.tensor.matmul(pts[0][:, 0 : 2 * P], dummy[:, 0:P],
                                 dummy[:], start=True, stop=True)

            # Payload: per tile, one matmul per 512-col half (its own
            # PSUM bank, single start/stop); ACT evacuates even banks,
            # DVE odd banks (bf16 cast).  Stores go PER BANK, right after
            # that bank's evac, alternating rings, so each store's ~1.9us
            # HWDGE issue latency overlaps the next bank's matmul/evac
            # instead of stacking after the whole tile.  (SWDGE prepared
            # stores were tried and are SLOWER: the Tile scheduler runs
            # the preps late and each drags a ~1.3us IncSwdgeSem.)
            for t in range(NT):
                for h in range(2):
                    k = 2 * t + h
                    cs = slice(t * B + h * CH, t * B + (h + 1) * CH)
                    nc.tensor.matmul(pts[k][:], v_sb[:],
                                     x_sbs[t][:, h * CH : (h + 1) * CH],
                                     start=True, stop=True)
                    if h == 0:
                        nc.scalar.copy(y_sb[:, cs], pts[k][:])
                        nc.sync.dma_start(y_out[:, cs], y_sb[:, cs])
                    else:
                        nc.vector.tensor_copy(y_sb[:, cs], pts[k][:])
                        nc.scalar.dma_start(y_out[:, cs], y_sb[:, cs])

    nc.finalize()
    return nc


_NC_CACHE = None


def _get_nc() -> bacc.Bacc:
    global _NC_CACHE
    if _NC_CACHE is None:
        _NC_CACHE = _build_nc()
    return _NC_CACHE


def _coef_values(M_re: np.ndarray, M_im: np.ndarray):
    """Host-side 2x2 expm of the anti-Hermitian generator -> V stationary.

    Returns (V, None): V is the [128, 128] bf16 kron(Q, I_32) stationary
    (second slot kept for interface compat with older harnesses).
    """
    M = M_re.astype(np.float64) + 1j * M_im.astype(np.float64)
    A = M - M.conj().T          # anti-Hermitian
    H = -1j * A                 # Hermitian
    w, Vec = np.linalg.eigh(H)
    Mexp = Vec @ np.diag(np.exp(1j * w)) @ Vec.conj().T   # expm(A), exact
    cr, ci = Mexp.real, Mexp.imag
    # out = V^T @ x with partition groups (x1re, x1im, x2re, x2im) and
    # output groups (o1re, o1im, o2re, o2im): V[p, i] = Q[p//32, i//32].
    Q = np.array([
        [cr[0, 0],  ci[0, 0],  cr[1, 0],  ci[1, 0]],
        [-ci[0, 0], cr[0, 0], -ci[1, 0],  cr[1, 0]],
        [cr[0, 1],  ci[0, 1],  cr[1, 1],  ci[1, 1]],
        [-ci[0, 1], cr[0, 1], -ci[1, 1],  cr[1, 1]],
    ], dtype=np.float32)
    V = np.kron(Q, np.eye(G, dtype=np.float32)).astype(NPBF)
    return V, None


def _in_map(x_re, x_im, V, cvec, d: int) -> dict:
    """Per-core input dict: pack the core's 128 pair-rows as 4 tiles of
    [128, 1024] with partition groups (x1re, x1im, x2re, x2im) x 32."""
    b1 = D // 2 + d * PROWS
    b2 = 3 * D // 4 + d * PROWS

    def grp(a, b0):
        return np.asarray(a[b0 : b0 + PROWS], dtype=NPBF).reshape(NT, G, B)

    # [NT, 4*G, B] -> [4*G, NT, B] -> [128, 4096]
    X = np.concatenate(
        [grp(x_re, b1), grp(x_im, b1), grp(x_re, b2), grp(x_im, b2)],
        axis=1,
    ).transpose(1, 0, 2).reshape(P, NT * B)
    return {"V": V, "X": np.ascontiguousarray(X)}


def kernel(M_re, M_im, x_re, x_im) -> np.ndarray:
    M_re = np.asarray(M_re, dtype=np.float32)
    M_im = np.asarray(M_im, dtype=np.float32)
    x_re = np.ascontiguousarray(x_re, dtype=np.float32)
    x_im = np.ascontiguousarray(x_im, dtype=np.float32)

    V, _ = _coef_values(M_re, M_im)
    in_maps = [_in_map(x_re, x_im, V, None, d) for d in range(NCORES)]

    nc = _get_nc()
    res = run_bass_kernel_spmd(nc, in_maps, core_ids=list(range(NCORES)))

    full = np.empty((D, B), dtype=np.complex64)
    # Identity block: assembled straight from the input during the gather.
    full.real[: D // 2] = x_re[: D // 2]
    full.imag[: D // 2] = x_im[: D // 2]
    for d, r in enumerate(res.results):
        b1 = D // 2 + d * PROWS
        b2 = 3 * D // 4 + d * PROWS
        Y = np.asarray(r["Y"]).reshape(P, NT, B).transpose(1, 0, 2)
        full.real[b1 : b1 + PROWS] = Y[:, 0 * G : 1 * G].reshape(PROWS, B)
        full.imag[b1 : b1 + PROWS] = Y[:, 1 * G : 2 * G].reshape(PROWS, B)
        full.real[b2 : b2 + PROWS] = Y[:, 2 * G : 3 * G].reshape(PROWS, B)
        full.imag[b2 : b2 + PROWS] = Y[:, 3 * G : 4 * G].reshape(PROWS, B)
    return full


# revision 15
# speedup vs baseline: 1.4244x; 1.0442x over previous
"""Trainium2 Bass kernel for the controlled-U (CU) gate application.

Math: the reference builds U = P0 (x) I (x) ... + P1 (x) Mexp (x) I ...
with dim=2, wires=12, index=(0,1), control_state=(1,). This factors as

    U = diag(I_2048, Mexp (x) I_1024)        (4096 x 4096)

so U @ x is:
    out[0:2048]     = x[0:2048]                        (identity)
    out[2048:3072]  = c00 * x[2048:3072] + c01 * x[3072:4096]
    out[3072:4096]  = c10 * x[2048:3072] + c11 * x[3072:4096]

with [[c00, c01], [c10, c11]] = Mexp = expm(M - M^H), a 2x2 unitary
computed exactly on host (eigendecomposition of the 2x2 Hermitian
generator).

The identity block is pure data movement with zero arithmetic, so it is
handled in the host-side gather: the top 2048 output rows are assembled
directly from the input array while interleaving to complex64 (the host
touches every element there anyway).  The device computes only the
non-trivial part -- the 2x2 complex mix over the bottom 2048 rows --
sharded row-wise across the 8 cores (128 pair-rows each).

Device-side formulation: each core packs its slice as four 128-partition
tiles where partitions carry 32-row groups of (x1_re, x1_im, x2_re,
x2_im) for paired rows (x1 from the c00/c01 block, x2 from the c10/c11
block).  The whole complex 2x2 mix is then ONE bf16 matmul per
512-column PSUM bank with a single constant stationary

    V = kron(Q, I_32),   Q[a, b] = coefficient of input group a
                                   in output group b

(out = V^T @ x contracts the partition dim, mixing re/im and the two
row blocks in one pass).  8 matmuls + 1 stationary load total; PSUM
banks are evacuated to bf16 SBUF split between the ACT and DVE engines,
and the 8 per-bank stores alternate the two HWDGE rings so loads and
stores overlap.  Per-core HBM traffic is ~2.06 MiB (vs 4.2 MiB when the
identity block rode the device), and the bf16 pipeline measures
~4e-3 rel err against the 2e-2 gate.
"""

import ml_dtypes
import numpy as np

import concourse.bacc as bacc
import concourse.mybir as mybir
from concourse.tile import TileContext
from concourse.bass_utils import run_bass_kernel_spmd

# Problem geometry (hardcoded per the task contract).
D = 4096           # state dimension 2**12
B = 1024           # batch
NCORES = 8
P = 128            # SBUF partitions
PROWS = D // 4 // NCORES   # 128 bottom pair rows per core
NT = 4             # tiles per core: 4 x [128, 1024]
CH = 512           # PSUM bank = 512 f32 columns
G = 32             # rows per partition group (4 groups of 32 = 128)
F32 = mybir.dt.float32
BF16 = mybir.dt.bfloat16
NPBF = ml_dtypes.bfloat16

N_WARM = 6         # dummy matmuls to bridge until the first tile lands


def _build_nc() -> bacc.Bacc:
    """Build the per-core Bass/Tile program (identical on all 8 cores)."""
    nc = bacc.Bacc("TRN2", enable_partition_id=False)

    v_in = nc.dram_tensor("V", [P, P], BF16, kind="ExternalInput")
    x_in = nc.dram_tensor("X", [P, NT * B], BF16, kind="ExternalInput")
    y_out = nc.dram_tensor("Y", [P, NT * B], BF16, kind="ExternalOutput")

    # Loads are issued RAW, before the TileContext: inside the context
    # they queue behind the all-engine barrier + body branch and only
    # start ~1 us into the exec window.  Emitted in the main block they
    # issue as soon as each ring's sequencer is up, and the transfers
    # fly while the engines are still in the entry barrier.  Completion
    # is signalled via manual semaphores (one per tensor) that the PE
    # waits on before consuming; the sems are cleared at program end so
    # re-executing the loaded NEFF sees them at zero again.
    v_sb = nc.alloc_sbuf_tensor("v_sb", [P, P], BF16)
    x_sbs = [nc.alloc_sbuf_tensor(f"x{t}_sb", [P, B], BF16)
             for t in range(NT)]
    s_v = nc.alloc_semaphore("s_v")
    s_x = [nc.alloc_semaphore(f"s_x{t}") for t in range(NT)]
    # Ring SP carries tiles 0/1, ring ACT carries V (tiny) then tiles
    # 2/3 -- DMA-pool arrival order == consumption order while both
    # rings' issue costs overlap.
    nc.sync.dma_start(x_sbs[0][:], x_in[:, 0 * B : 1 * B]).then_inc(s_x[0], 16)
    nc.scalar.dma_start(v_sb[:], v_in[:]).then_inc(s_v, 16)
    nc.sync.dma_start(x_sbs[1][:], x_in[:, 1 * B : 2 * B]).then_inc(s_x[1], 16)
    nc.scalar.dma_start(x_sbs[2][:], x_in[:, 2 * B : 3 * B]).then_inc(s_x[2], 16)
    nc.scalar.dma_start(x_sbs[3][:], x_in[:, 3 * B : 4 * B]).then_inc(s_x[3], 16)

    with TileContext(nc) as tc:
        with (
            tc.tile_pool(name="io", bufs=1) as io_pool,
            tc.tile_pool(name="scr", bufs=1) as scr_pool,
            tc.tile_pool(name="psum", bufs=1, space="PSUM") as psum_pool,
        ):
            y_sb = io_pool.tile([P, NT * B], BF16, tag="y")

            # PE warmup: the HAM clock gate runs the PE at 1.2 GHz until
            # it has seen ~4 us of sustained activity; dummy matmuls on a
            # memset tile bridge the load-latency window (first tile is
            # consumable ~4 us into the body) so payload matmuls run
            # closer to 2.4 GHz.  They write a payload PSUM bank --
            # harmless, the payload matmul resets it with start=True.
            dummy = scr_pool.tile([P, 2 * P], BF16, tag="dummy")
            nc.gpsimd.memset(dummy[:], 0.0)

            pts = [psum_pool.tile([P, CH], F32, tag=f"ps{k}", name=f"ps{k}")
                   for k in range(2 * NT)]
            for _ in range(N_WARM):
                nc.tensor.matmul(pts[0][:, 0 : 2 * P], dummy[:, 0:P],
                                 dummy[:], start=True, stop=True)

            # Payload: per tile, one matmul per 512-col half (its own
            # PSUM bank, single start/stop); ACT evacuates even banks,
            # DVE odd banks (bf16 cast).  Stores go PER BANK, right after
            # that bank's evac, alternating rings, so each store's ~1.9us
            # HWDGE issue latency overlaps the next bank's matmul/evac
            # instead of stacking after the whole tile.  (SWDGE prepared
            # stores were tried and are SLOWER: the Tile scheduler runs
            # the preps late and each drags a ~1.3us IncSwdgeSem.)
            nc.tensor.wait_ge(s_v, 16)
            for t in range(NT):
                nc.tensor.wait_ge(s_x[t], 16)
                for h in range(2):
                    k = 2 * t + h
                    cs = slice(t * B + h * CH, t * B + (h + 1) * CH)
                    nc.tensor.matmul(pts[k][:], v_sb[:],
                                     x_sbs[t][:, h * CH : (h + 1) * CH],
                                     start=True, stop=True)
                    if h == 0:
                        nc.scalar.copy(y_sb[:, cs], pts[k][:])
                        nc.sync.dma_start(y_out[:, cs], y_sb[:, cs])
                    else:
                        nc.vector.tensor_copy(y_sb[:, cs], pts[k][:])
                        nc.scalar.dma_start(y_out[:, cs], y_sb[:, cs])

    nc.gpsimd.sem_clear(s_v)
    for s in s_x:
        nc.gpsimd.sem_clear(s)

    nc.finalize()
    return nc


_NC_CACHE = None


def _get_nc() -> bacc.Bacc:
    global _NC_CACHE
    if _NC_CACHE is None:
        _NC_CACHE = _build_nc()
    return _NC_CACHE


def _coef_values(M_re: np.ndarray, M_im: np.ndarray):
    """Host-side 2x2 expm of the anti-Hermitian generator -> V stationary.

    Returns (V, None): V is the [128, 128] bf16 kron(Q, I_32) stationary
    (second slot kept for interface compat with older harnesses).
    """
    M = M_re.astype(np.float64) + 1j * M_im.astype(np.float64)
    A = M - M.conj().T          # anti-Hermitian
    H = -1j * A                 # Hermitian
    w, Vec = np.linalg.eigh(H)
    Mexp = Vec @ np.diag(np.exp(1j * w)) @ Vec.conj().T   # expm(A), exact
    cr, ci = Mexp.real, Mexp.imag
    # out = V^T @ x with partition groups (x1re, x1im, x2re, x2im) and
    # output groups (o1re, o1im, o2re, o2im): V[p, i] = Q[p//32, i//32].
    Q = np.array([
        [cr[0, 0],  ci[0, 0],  cr[1, 0],  ci[1, 0]],
        [-ci[0, 0], cr[0, 0], -ci[1, 0],  cr[1, 0]],
        [cr[0, 1],  ci[0, 1],  cr[1, 1],  ci[1, 1]],
        [-ci[0, 1], cr[0, 1], -ci[1, 1],  cr[1, 1]],
    ], dtype=np.float32)
    V = np.kron(Q, np.eye(G, dtype=np.float32)).astype(NPBF)
    return V, None


def _in_map(x_re, x_im, V, cvec, d: int) -> dict:
    """Per-core input dict: pack the core's 128 pair-rows as 4 tiles of
    [128, 1024] with partition groups (x1re, x1im, x2re, x2im) x 32."""
    b1 = D // 2 + d * PROWS
    b2 = 3 * D // 4 + d * PROWS

    def grp(a, b0):
        return np.asarray(a[b0 : b0 + PROWS], dtype=NPBF).reshape(NT, G, B)

    # [NT, 4*G, B] -> [4*G, NT, B] -> [128, 4096]; V rides in front.
    X = np.concatenate(
        [grp(x_re, b1), grp(x_im, b1), grp(x_re, b2), grp(x_im, b2)],
        axis=1,
    ).transpose(1, 0, 2).reshape(P, NT * B)
    return {"X": np.ascontiguousarray(np.concatenate([V, X], axis=1))}


def kernel(M_re, M_im, x_re, x_im) -> np.ndarray:
    M_re = np.asarray(M_re, dtype=np.float32)
    M_im = np.asarray(M_im, dtype=np.float32)
    x_re = np.ascontiguousarray(x_re, dtype=np.float32)
    x_im = np.ascontiguousarray(x_im, dtype=np.float32)

    V, _ = _coef_values(M_re, M_im)
    in_maps = [_in_map(x_re, x_im, V, None, d) for d in range(NCORES)]

    nc = _get_nc()
    res = run_bass_kernel_spmd(nc, in_maps, core_ids=list(range(NCORES)))

    full = np.empty((D, B), dtype=np.complex64)
    # Identity block: assembled straight from the input during the gather.
    full.real[: D // 2] = x_re[: D // 2]
    full.imag[: D // 2] = x_im[: D // 2]
    for d, r in enumerate(res.results):
        b1 = D // 2 + d * PROWS
        b2 = 3 * D // 4 + d * PROWS
        Y = np.asarray(r["Y"]).reshape(P, NT, B).transpose(1, 0, 2)
        full.real[b1 : b1 + PROWS] = Y[:, 0 * G : 1 * G].reshape(PROWS, B)
        full.imag[b1 : b1 + PROWS] = Y[:, 1 * G : 2 * G].reshape(PROWS, B)
        full.real[b2 : b2 + PROWS] = Y[:, 2 * G : 3 * G].reshape(PROWS, B)
        full.imag[b2 : b2 + PROWS] = Y[:, 3 * G : 4 * G].reshape(PROWS, B)
    return full


# revision 16
# speedup vs baseline: 1.4730x; 1.0341x over previous
"""Trainium2 Bass kernel for the controlled-U (CU) gate application.

Math: the reference builds U = P0 (x) I (x) ... + P1 (x) Mexp (x) I ...
with dim=2, wires=12, index=(0,1), control_state=(1,). This factors as

    U = diag(I_2048, Mexp (x) I_1024)        (4096 x 4096)

so U @ x is:
    out[0:2048]     = x[0:2048]                        (identity)
    out[2048:3072]  = c00 * x[2048:3072] + c01 * x[3072:4096]
    out[3072:4096]  = c10 * x[2048:3072] + c11 * x[3072:4096]

with [[c00, c01], [c10, c11]] = Mexp = expm(M - M^H), a 2x2 unitary
computed exactly on host (eigendecomposition of the 2x2 Hermitian
generator).

The identity block is pure data movement with zero arithmetic, so it is
handled in the host-side gather: the top 2048 output rows are assembled
directly from the input array while interleaving to complex64 (the host
touches every element there anyway).  The device computes only the
non-trivial part -- the 2x2 complex mix over the bottom 2048 rows --
sharded row-wise across the 8 cores (128 pair-rows each).

Device-side formulation: each core packs its slice as four 128-partition
tiles where partitions carry 32-row groups of (x1_re, x1_im, x2_re,
x2_im) for paired rows (x1 from the c00/c01 block, x2 from the c10/c11
block).  The whole complex 2x2 mix is then ONE bf16 matmul per
512-column PSUM bank with a single constant stationary

    V = kron(Q, I_32),   Q[a, b] = coefficient of input group a
                                   in output group b

(out = V^T @ x contracts the partition dim, mixing re/im and the two
row blocks in one pass).  8 matmuls + 1 stationary load total; PSUM
banks are evacuated to bf16 SBUF split between the ACT and DVE engines,
and the 8 per-bank stores alternate the two HWDGE rings so loads and
stores overlap.  Per-core HBM traffic is ~2.06 MiB (vs 4.2 MiB when the
identity block rode the device), and the bf16 pipeline measures
~4e-3 rel err against the 2e-2 gate.
"""

import ml_dtypes
import numpy as np

import concourse.bacc as bacc
import concourse.mybir as mybir
from concourse.tile import TileContext
from concourse.bass_utils import run_bass_kernel_spmd

# Problem geometry (hardcoded per the task contract).
D = 4096           # state dimension 2**12
B = 1024           # batch
NCORES = 8
P = 128            # SBUF partitions
PROWS = D // 4 // NCORES   # 128 bottom pair rows per core
NT = 4             # tiles per core: 4 x [128, 1024]
CH = 512           # PSUM bank = 512 f32 columns
G = 32             # rows per partition group (4 groups of 32 = 128)
F32 = mybir.dt.float32
BF16 = mybir.dt.bfloat16
NPBF = ml_dtypes.bfloat16

N_WARM = 6         # dummy matmuls to bridge until the first tile lands


def _build_nc() -> bacc.Bacc:
    """Build the per-core raw-bass program (identical on all 8 cores).

    Raw bass (no TileContext), manual semaphores throughout.  The input
    is one stream [V | 4096 cols] split into UNEVEN load groups --
    [V+banks 0-2], [banks 3-4], [banks 5-6], [bank 7] -- so the last
    load (which the DMA pool serves last, total wire time being fixed)
    feeds only a single bank: half the matmul/evac/store tail of an
    even split.  Bank 7 is additionally evacuated as two 256-col halves
    by ACT and DVE in parallel.  PE dummy matmuls bridge every idle gap
    so the clock gate never re-throttles.
    """
    nc = bacc.Bacc("TRN2", enable_partition_id=False)

    x_in = nc.dram_tensor("X", [P, P + NT * B], BF16, kind="ExternalInput")
    y_out = nc.dram_tensor("Y", [P, NT * B], BF16, kind="ExternalOutput")

    NB = 2 * NT                   # 8 banks of 512 columns
    vx_sb = nc.alloc_sbuf_tensor("vx_sb", [P, P + 3 * CH], BF16)
    xb34 = nc.alloc_sbuf_tensor("xb34_sb", [P, 2 * CH], BF16)
    xb56 = nc.alloc_sbuf_tensor("xb56_sb", [P, 2 * CH], BF16)
    xb7 = nc.alloc_sbuf_tensor("xb7_sb", [P, CH], BF16)
    v_sb = vx_sb[:, 0:P]
    xbs = [vx_sb[:, P + i * CH : P + (i + 1) * CH] for i in range(3)] + \
          [xb34[:, 0:CH], xb34[:, CH : 2 * CH],
           xb56[:, 0:CH], xb56[:, CH : 2 * CH], xb7[:, 0:CH]]
    y_sb = nc.alloc_sbuf_tensor("y_sb", [P, NT * B], BF16)
    dummy = nc.alloc_sbuf_tensor("dummy_sb", [P, 2 * P], BF16)
    pts = [nc.alloc_psum_tensor(f"ps{k}", [P, CH], F32) for k in range(NB)]

    s_l = [nc.alloc_semaphore(f"s_l{g}") for g in range(4)]
    s_dum = nc.alloc_semaphore("s_dum")
    s_mm = [nc.alloc_semaphore(f"s_mm{k}") for k in range(NB)]
    s_ev = [nc.alloc_semaphore(f"s_ev{k}") for k in range(NB)]
    s_sp = nc.alloc_semaphore("s_sp")
    s_act = nc.alloc_semaphore("s_act")
    all_sems = [*s_l, s_dum, *s_mm, *s_ev, s_sp, s_act]

    # Loads at the very top of main (transfers fly while engines boot):
    # SP: [V|b0-b2] then [b5,b6]; ACT: [b3,b4] then [b7].
    c0 = P + 3 * CH
    nc.sync.dma_start(vx_sb[:], x_in[:, 0:c0]).then_inc(s_l[0], 16)
    nc.scalar.dma_start(xb34[:],
                        x_in[:, c0 : c0 + 2 * CH]).then_inc(s_l[1], 16)
    nc.sync.dma_start(xb56[:],
                      x_in[:, c0 + 2 * CH : c0 + 4 * CH]).then_inc(s_l[2], 16)
    nc.scalar.dma_start(xb7[:],
                        x_in[:, c0 + 4 * CH : c0 + 5 * CH]).then_inc(s_l[3], 16)

    # PE: warm, then per-group payload with gap dummies (targeting a
    # LATER bank's psum -- its real matmul resets it with start=True).
    nc.gpsimd.memset(dummy[:], 0.0).then_inc(s_dum, 1)
    nc.tensor.wait_ge(s_dum, 1)
    for _ in range(N_WARM):
        nc.tensor.matmul(pts[0][:, 0 : 2 * P], dummy[:, 0:P], dummy[:],
                         start=True, stop=True)

    groups = [(s_l[0], (0, 1, 2)), (s_l[1], (3, 4)),
              (s_l[2], (5, 6)), (s_l[3], (7,))]
    gap_bank = {0: 4, 2: 7}           # fillers after groups 0 and 2
    gap_n = {0: N_GAP, 2: N_GAP2}
    for g, (sem, banks) in enumerate(groups):
        nc.tensor.wait_ge(sem, 16)
        for k in banks:
            nc.tensor.matmul(pts[k][:], v_sb, xbs[k],
                             start=True, stop=True).then_inc(s_mm[k], 1)
        if g in gap_bank:
            for _ in range(gap_n[g]):
                nc.tensor.matmul(pts[gap_bank[g]][:, 0 : 2 * P],
                                 dummy[:, 0:P], dummy[:],
                                 start=True, stop=True)

    # Evacs + stores, per bank: ACT evacuates even banks, DVE odd banks
    # 1-5; bank 7 is split into two 256-col halves (ACT + DVE parallel,
    # s_ev[7] reaches 2).  Stores ride SP (even) / ACT (odd) right after
    # their bank's evac so the ~1.9us HWDGE issue latency overlaps the
    # next bank's work instead of stacking on the tail.
    def col(k):
        return slice(k * CH, (k + 1) * CH)

    for k in (0, 2, 4, 6):             # ACT queue: e0,s1,e2,s3,e4,s5,e6
        nc.scalar.wait_ge(s_mm[k], 1)
        nc.scalar.copy(y_sb[:, col(k)], pts[k][:]).then_inc(s_ev[k], 1)
        if k < 6:
            nc.scalar.wait_ge(s_ev[k + 1], 1)
            nc.scalar.dma_start(y_out[:, col(k + 1)],
                                y_sb[:, col(k + 1)]).then_inc(s_act, 16)
    for k in (1, 3, 5):                # DVE queue: e1, e3, e5
        nc.vector.wait_ge(s_mm[k], 1)
        nc.vector.tensor_copy(y_sb[:, col(k)], pts[k][:]).then_inc(s_ev[k], 1)
    # bank 7 half-evacs, both engines in parallel
    nc.scalar.wait_ge(s_mm[7], 1)
    nc.scalar.copy(y_sb[:, 7 * CH : 7 * CH + CH // 2],
                   pts[7][:, 0 : CH // 2]).then_inc(s_ev[7], 1)
    nc.vector.wait_ge(s_mm[7], 1)
    nc.vector.tensor_copy(y_sb[:, 7 * CH + CH // 2 : 8 * CH],
                          pts[7][:, CH // 2 : CH]).then_inc(s_ev[7], 1)
    nc.scalar.wait_ge(s_ev[7], 2)      # ACT queue tail: s7
    nc.scalar.dma_start(y_out[:, col(7)],
                        y_sb[:, col(7)]).then_inc(s_act, 16)
    for k in (0, 2, 4, 6):             # SP queue: s0, s2, s4, s6
        nc.sync.wait_ge(s_ev[k], 1)
        nc.sync.dma_start(y_out[:, col(k)],
                          y_sb[:, col(k)]).then_inc(s_sp, 16)

    # Completion: each ring waits its own 4 store DMAs, then a barrier
    # (so no clear can race a pending wait), then reset every semaphore
    # we touched so re-executing the loaded NEFF starts from zero.
    nc.sync.wait_ge(s_sp, 64)
    nc.scalar.wait_ge(s_act, 64)
    nc.all_engine_barrier()
    nums = sorted(s.num for s in all_sems)
    i = 0
    while i < len(nums):
        j = i
        while j + 1 < len(nums) and nums[j + 1] == nums[j] + 1:
            j += 1
        nc.gpsimd.sem_clear(range(nums[i], nums[j] + 1))
        i = j + 1
    nc.all_engine_barrier()

    nc.finalize()
    return nc


_NC_CACHE = None


def _get_nc() -> bacc.Bacc:
    global _NC_CACHE
    if _NC_CACHE is None:
        _NC_CACHE = _build_nc()
    return _NC_CACHE


def _coef_values(M_re: np.ndarray, M_im: np.ndarray):
    """Host-side 2x2 expm of the anti-Hermitian generator -> V stationary.

    Returns (V, None): V is the [128, 128] bf16 kron(Q, I_32) stationary
    (second slot kept for interface compat with older harnesses).
    """
    M = M_re.astype(np.float64) + 1j * M_im.astype(np.float64)
    A = M - M.conj().T          # anti-Hermitian
    H = -1j * A                 # Hermitian
    w, Vec = np.linalg.eigh(H)
    Mexp = Vec @ np.diag(np.exp(1j * w)) @ Vec.conj().T   # expm(A), exact
    cr, ci = Mexp.real, Mexp.imag
    # out = V^T @ x with partition groups (x1re, x1im, x2re, x2im) and
    # output groups (o1re, o1im, o2re, o2im): V[p, i] = Q[p//32, i//32].
    Q = np.array([
        [cr[0, 0],  ci[0, 0],  cr[1, 0],  ci[1, 0]],
        [-ci[0, 0], cr[0, 0], -ci[1, 0],  cr[1, 0]],
        [cr[0, 1],  ci[0, 1],  cr[1, 1],  ci[1, 1]],
        [-ci[0, 1], cr[0, 1], -ci[1, 1],  cr[1, 1]],
    ], dtype=np.float32)
    V = np.kron(Q, np.eye(G, dtype=np.float32)).astype(NPBF)
    return V, None


def _in_map(x_re, x_im, V, cvec, d: int) -> dict:
    """Per-core input dict: pack the core's 128 pair-rows as 4 tiles of
    [128, 1024] with partition groups (x1re, x1im, x2re, x2im) x 32."""
    b1 = D // 2 + d * PROWS
    b2 = 3 * D // 4 + d * PROWS

    def grp(a, b0):
        return np.asarray(a[b0 : b0 + PROWS], dtype=NPBF).reshape(NT, G, B)

    # [NT, 4*G, B] -> [4*G, NT, B] -> [128, 4096]; V rides in front.
    X = np.concatenate(
        [grp(x_re, b1), grp(x_im, b1), grp(x_re, b2), grp(x_im, b2)],
        axis=1,
    ).transpose(1, 0, 2).reshape(P, NT * B)
    return {"X": np.ascontiguousarray(np.concatenate([V, X], axis=1))}


def kernel(M_re, M_im, x_re, x_im) -> np.ndarray:
    M_re = np.asarray(M_re, dtype=np.float32)
    M_im = np.asarray(M_im, dtype=np.float32)
    x_re = np.ascontiguousarray(x_re, dtype=np.float32)
    x_im = np.ascontiguousarray(x_im, dtype=np.float32)

    V, _ = _coef_values(M_re, M_im)
    in_maps = [_in_map(x_re, x_im, V, None, d) for d in range(NCORES)]

    nc = _get_nc()
    res = run_bass_kernel_spmd(nc, in_maps, core_ids=list(range(NCORES)))

    full = np.empty((D, B), dtype=np.complex64)
    # Identity block: assembled straight from the input during the gather.
    full.real[: D // 2] = x_re[: D // 2]
    full.imag[: D // 2] = x_im[: D // 2]
    for d, r in enumerate(res.results):
        b1 = D // 2 + d * PROWS
        b2 = 3 * D // 4 + d * PROWS
        Y = np.asarray(r["Y"]).reshape(P, NT, B).transpose(1, 0, 2)
        full.real[b1 : b1 + PROWS] = Y[:, 0 * G : 1 * G].reshape(PROWS, B)
        full.imag[b1 : b1 + PROWS] = Y[:, 1 * G : 2 * G].reshape(PROWS, B)
        full.real[b2 : b2 + PROWS] = Y[:, 2 * G : 3 * G].reshape(PROWS, B)
        full.imag[b2 : b2 + PROWS] = Y[:, 3 * G : 4 * G].reshape(PROWS, B)
    return full
